# revision 30
# baseline (speedup 1.0000x reference)
"""DeepSeek sparse attention (single-query, MQA low-rank KV) on 8 trn2 cores.

Strategy (data-parallel: batch b -> core b):
  Launch 1 (device): indexer scores for the last query, folded to a GEMV:
      w_b = Wkv_down[:, :L] @ q_idx[b]  (host, per batch), both x and w_b
      quantized to fp8 e4m3 -> scores[s] = x8[s,:] . w8  on the PE with
      x8 chunks as the stationary operand and w8 columns as the 1-wide
      moving operand.  x is streamed in s-block slabs so the PE trails
      the DMA by one slab; the fp8 x DMA (16.8 MB) is the whole cost.
  Host: noisy-score top-k certain/band split (margin 768 ranks >2x the
      measured 358-rank worst-case displacement of the fp8 GEMV scores
      vs the reference's fp8-emulated scores); band rows rescored
      bit-exactly vs the reference via jax-CPU slice gemm; exact top-k
      set.  Selection must be exact: one swapped token moves the output
      by ~1.7e-2 relative, right at the tolerance.
  Launch 2 (device): doubly-absorbed MLA attention over the selected
      tokens in fp16.  The K path is folded through the query on the
      host twice: q~[h] = q[h] @ Wk_up[:,h]^T / sqrt(dh) and
      w^[h] = Wkv_down_K @ q~[h], so NEITHER the K down-projection nor
      the K up-projection ever runs on the device -- logits[h,k] are a
      direct contraction w^[h] . x_sel[k] (16 matmuls per k-block).
      The only big GEMM left is the V-latent down-projection
      x_sel @ Wkv_down_V (4 psum banks, no drain stalls).  Softmax runs
      without max-subtraction (logits ~N(0, 0.4), bounded ~+-3); exp for
      k-block i hides under block i+1's V pass.  o via per-head Wv_up,
      out-projection with 1-wide moving columns.  K-path biases shift
      logits per-head uniformly and cancel exactly in softmax; all
      V-side biases are additive constants on o (sum of attn = 1) and
      fold into the host output bias (bkv_down_V @ Wv_up + bv_up) @
      Wout + bout.  All big operands are host-relayouted so each lands
      as one contiguous DMA; x slabs get dedicated buffers and are
      issued dep-free ahead of the 23 us Wout transfer.

Shapes hardcoded: B=8, S=8192, D=2048, H=16, dh=128, L=512, k=2048.
"""
import numpy as np
import ml_dtypes

import concourse.bacc as bacc
import concourse.tile as tile
import concourse.mybir as mybir
from concourse import masks
from concourse.bass_utils import run_bass_kernel_spmd

F8NP = ml_dtypes.float8_e4m3fn
F16NP = np.float16
dt = mybir.dt

B, S, D = 8, 8192, 2048
H, DH, L = 16, 128, 512
TOPK = 2048
MARGIN = 768
NCORES = 8
RSQ = float(1.0 / np.sqrt(np.float32(DH)))  # 1/sqrt(128)

ND = D // 128      # 16 d-chunks
NLT = L // 128     # 4 K-latent l-tiles
NKT = TOPK // 128  # 16 k-tiles
NKB = TOPK // 512  # 4 k-blocks
NST = S // 128     # 64 s-tiles
NSB = S // 512     # 16 s-slabs

_STATE = {}
LAST_EXEC = {}


# ---------------------------------------------------------------- launch 1
def _build_l1():
    """scores[s] = sum_d x8[d, s] * w8[d], fp8, one psum column per s-tile."""
    nc = bacc.Bacc("TRN2", target_bir_lowering=False, debug=False,
                   num_devices=NCORES)
    # x8S is host-relayouted slab-major: row i*128+p, col d*512+s holds
    # x[i*512+s, d*128+p], so slab i is one contiguous [128, 8192] DMA.
    x8S = nc.dram_tensor("x8S", [NSB * 128, ND * 512], dt.float8e4,
                         kind="ExternalInput").ap()
    w8 = nc.dram_tensor("w8", [128, ND], dt.float8e4, kind="ExternalInput").ap()
    sc = nc.dram_tensor("sc", [128, NST], dt.float32, kind="ExternalOutput").ap()

    with tile.TileContext(nc) as tc:
        with (
            tc.tile_pool(name="p", bufs=1) as p,
            tc.tile_pool(name="xp", bufs=1) as xp,
            tc.tile_pool(name="ps", bufs=1, space="PSUM") as psp,
        ):
            w = p.tile([128, ND], dt.float8e4)
            nc.sync.dma_start(w[:], w8)
            pst = psp.tile([128, 512], dt.float32)
            sb = p.tile([128, NST], dt.float32)
            xsl = [None] * NSB
            for i in range(NSB):
                # slab i: all 16 d-chunks of s-columns [i*512, (i+1)*512);
                # the last slab lands in 4 pieces to shorten the tail chain
                xsl[i] = xp.tile([128, ND * 512], dt.float8e4,
                                 tag=f"x{i % 3}", name=f"xsl_{i}")
                if i < NSB - 1:
                    nc.sync.dma_start(xsl[i][:], x8S[i * 128:(i + 1) * 128, :])
                else:
                    for q in range(4):
                        nc.sync.dma_start(
                            xsl[i][:, q * 2048:(q + 1) * 2048],
                            x8S[i * 128:(i + 1) * 128, q * 2048:(q + 1) * 2048])
                for st in range(4):
                    s = i * 4 + st
                    for d in range(ND):
                        nc.tensor.matmul(
                            pst[:, s:s + 1],
                            xsl[i][:, d * 512 + st * 128:
                                    d * 512 + (st + 1) * 128],
                            w[:, d:d + 1], start=(d == 0), stop=(d == ND - 1))
                if i % 4 == 3:
                    # drain per quarter into sb; a single output DMA at the
                    # end (interleaved per-quarter DMAs steal serialized
                    # HWDGE/DMA-engine slots from the x stream, ~1.7 us)
                    q = i // 4
                    nc.scalar.copy(sb[:, q * 16:(q + 1) * 16],
                                   pst[:, q * 16:(q + 1) * 16])
            nc.sync.dma_start(sc, sb[:])
    nc.compile()
    return nc


# ---------------------------------------------------------------- launch 2
def _build_l2():
    f16 = dt.float16
    nc = bacc.Bacc("TRN2", target_bir_lowering=False, debug=False,
                   num_devices=NCORES)
    # all big operands are host-relayouted so each is one (or a few)
    # contiguous [128, N] DMAs: row p, col-major exactly as the SBUF tile.
    xselS = nc.dram_tensor("xselS", [NKB * 128, ND * 512], f16,
                           kind="ExternalInput").ap()
    wdkVf = nc.dram_tensor("wdkVf", [128, ND * L], f16,
                           kind="ExternalInput").ap()
    wvupf = nc.dram_tensor("wvupf", [128, NLT * D], f16,
                           kind="ExternalInput").ap()
    woutf = nc.dram_tensor("woutf", [128, ND * D], f16,
                           kind="ExternalInput").ap()
    what = nc.dram_tensor("what", [128, ND * H], f16, kind="ExternalInput").ap()
    boutc = nc.dram_tensor("boutc", [128, ND], dt.float32,
                           kind="ExternalInput").ap()
    outc = nc.dram_tensor("outc", [128, ND], dt.float32,
                          kind="ExternalOutput").ap()

    with tile.TileContext(nc) as tc:
        with (
            tc.tile_pool(name="cst", bufs=1) as cst,
            tc.tile_pool(name="wp", bufs=1) as wp,
            tc.tile_pool(name="xp", bufs=1) as xp,
        ):
            whatsb = cst.tile([128, ND * H], f16)
            nc.sync.dma_start(whatsb[:], what)
            boutcsb = cst.tile([128, ND], dt.float32)
            nc.sync.dma_start(boutcsb[:], boutc)
            idf32 = cst.tile([16, 16], dt.float32)
            masks.make_identity(nc, idf32[:])
            idf16 = cst.tile([16, 16], f16)
            nc.scalar.copy(idf16[:], idf32[:])

            wdkVsb = wp.tile([128, ND * L], f16)
            vs = wp.tile([128, NKT * L], f16)       # [k, l] 16 tiles
            wvupsb = wp.tile([128, NLT * D], f16)
            woutsb = wp.tile([128, ND * D], f16)
            den4 = cst.tile([16, 4], dt.float32)
            den = cst.tile([16, 1], dt.float32)
            rden = cst.tile([16, 1], dt.float32)
            attnE = cst.tile([16, TOPK], f16)
            attnN = cst.tile([16, TOPK], f16)

            # head DMAs: wdkV and the first x slab land as interleaved
            # 2-chunk column slices so the V down-projection starts early
            # and stays DMA-paced.
            xsl = [None] * NKB
            xsl[0] = xp.tile([128, ND * 512], f16, tag="xsl0", name="xsl_0")
            for j in range(8):
                nc.sync.dma_start(wdkVsb[:, j * 1024:(j + 1) * 1024],
                                  wdkVf[:, j * 1024:(j + 1) * 1024])
                nc.sync.dma_start(xsl[0][:, j * 1024:(j + 1) * 1024],
                                  xselS[0:128, j * 1024:(j + 1) * 1024])

            # ---- down-projection + per-block logits/exp
            with (
                tc.tile_pool(name="vps", bufs=1, space="PSUM") as vps,
                tc.tile_pool(name="lp", bufs=1, space="PSUM") as lp,
            ):
                for kb in range(NKB):
                    # prefetch: all remaining slabs get their own buffers
                    # and are issued dep-free in k-block 0, so none of them
                    # queues behind the 23 us Wout transfer; weights follow.
                    if kb == 0:
                        for nb in range(1, NKB):
                            xsl[nb] = xp.tile([128, ND * 512], f16,
                                              tag=f"xsl{nb}",
                                              name=f"xsl_{nb}")
                            nc.sync.dma_start(
                                xsl[nb][:],
                                xselS[nb * 128:(nb + 1) * 128, :])
                    if kb == 1:
                        nc.sync.dma_start(wvupsb[:], wvupf)
                    if kb == 2:
                        nc.sync.dma_start(woutsb[:], woutf)

                    # absorbed logits: logit[h,k] = w^[h] . x_sel[k] with
                    # w^ = Wkv_down_K @ q~ folded on the host -- the K
                    # down-projection never runs on the device at all.
                    # exp overlaps the next block's V pass.
                    def _logits(kb):
                        lps = lp.tile([128, 512], dt.float32, tag="l",
                                      name=f"lps{kb}")
                        for d in range(ND):
                            nc.tensor.matmul(
                                lps[:H, :],
                                whatsb[:, d * H:(d + 1) * H],
                                xsl[kb][:, d * 512:(d + 1) * 512],
                                start=(d == 0), stop=(d == ND - 1))
                        nc.scalar.activation(
                            attnE[:, kb * 512:(kb + 1) * 512], lps[:H, :],
                            mybir.ActivationFunctionType.Exp,
                            accum_out=den4[:, kb:kb + 1])

                    pvs = [vps.tile([128, 512], dt.float32, tag=f"v{i}",
                                    name=f"pv{kb}_{i}")
                           for i in range(4)]
                    for d in range(ND):
                        for ktl in range(4):
                            nc.tensor.matmul(
                                pvs[ktl][:],
                                xsl[kb][:, d * 512 + ktl * 128:
                                        d * 512 + (ktl + 1) * 128],
                                wdkVsb[:, d * L:(d + 1) * L],
                                start=(d == 0), stop=(d == ND - 1))
                    for ktl in range(4):
                        kt = kb * 4 + ktl
                        nc.vector.tensor_copy(
                            vs[:, kt * L:(kt + 1) * L], pvs[ktl][:])
                    _logits(kb)

            nc.vector.reduce_sum(den[:], den4[:], axis=mybir.AxisListType.X)
            nc.vector.reciprocal(rden[:], den[:])
            nc.vector.tensor_scalar_mul(attnN[:], attnE[:], rden[:])

            # ---- oT~[l,h] = sum_k V[k,l] attn[h,k] via attn^T tiles
            attnT = cst.tile([128, NKT * H], f16)
            otsb = cst.tile([128, NLT * H], f16)
            with (
                tc.tile_pool(name="tps", bufs=1, space="PSUM") as tps,
                tc.tile_pool(name="otp", bufs=1, space="PSUM") as otp,
            ):
                for g in range(4):
                    ptt = tps.tile([128, 64], f16, tag=f"t{g % 2}",
                                   name=f"ptt{g}")
                    for i in range(4):
                        kt = g * 4 + i
                        nc.tensor.matmul(
                            ptt[:, i * 16:(i + 1) * 16],
                            attnN[:, kt * 128:(kt + 1) * 128],
                            idf16[:], is_transpose=True)
                    nc.scalar.copy(attnT[:, g * 64:(g + 1) * 64], ptt[:])
                ots = [otp.tile([128, H], dt.float32, tag=f"o{lc}",
                                name=f"ot{lc}")
                       for lc in range(NLT)]
                for kt in range(NKT):
                    for lc in range(NLT):
                        nc.tensor.matmul(
                            ots[lc][:],
                            vs[:, kt * L + lc * 128: kt * L + (lc + 1) * 128],
                            attnT[:, kt * H:(kt + 1) * H],
                            start=(kt == 0), stop=(kt == NKT - 1))
                for lc in range(NLT):
                    nc.scalar.copy(otsb[:, lc * H:(lc + 1) * H], ots[lc][:])

            # ---- oT[dh,h] = sum_l Wv_up[l, h-block] oT~[l, h], 4 heads
            # per psum tile; the valid column for head h is (h%4)*16 + h.
            oTbig = cst.tile([128, 4 * 64], f16)
            with tc.tile_pool(name="hp", bufs=1, space="PSUM") as hp:
                for hb in range(4):
                    ph = hp.tile([128, 64], dt.float32, tag=f"h{hb % 2}",
                                 name=f"ph{hb}")
                    for i in range(4):
                        h = hb * 4 + i
                        for lc in range(NLT):
                            nc.tensor.matmul(
                                ph[:, i * 16:(i + 1) * 16],
                                wvupsb[:, lc * D + h * DH:
                                       lc * D + (h + 1) * DH],
                                otsb[:, lc * H:(lc + 1) * H],
                                start=(lc == 0), stop=(lc == NLT - 1))
                    nc.scalar.copy(oTbig[:, hb * 64:(hb + 1) * 64], ph[:])

            # ---- out-projection: outc[n%128, n//128] = sum_d o[d] Wout[d,n]
            outcsb = cst.tile([128, ND], dt.float32)
            with tc.tile_pool(name="cp", bufs=1, space="PSUM") as cp:
                pc = cp.tile([128, ND], dt.float32)
                for j in range(ND):
                    for dc in range(ND):
                        col = (dc // 4) * 64 + (dc % 4) * 16 + dc
                        nc.tensor.matmul(
                            pc[:, j:j + 1],
                            woutsb[:, dc * D + j * 128: dc * D + (j + 1) * 128],
                            oTbig[:, col:col + 1],
                            start=(dc == 0), stop=(dc == ND - 1))
                nc.vector.tensor_add(outcsb[:], pc[:], boutcsb[:])
            nc.sync.dma_start(outc, outcsb[:])
    nc.compile()
    return nc


# ---------------------------------------------------------------- timing
def model_time(nc):
    """Cost-model (TimelineSim) estimate in ns for one core."""
    from concourse.timeline_sim import TimelineSim
    return TimelineSim(nc).simulate()


def _run_spmd_retry(nc, in_maps, cores, trace=False):
    """One retry: a previously crashed process can leave the device in a
    transient NRT_EXEC_UNIT_UNRECOVERABLE state that clears on re-run."""
    try:
        return run_bass_kernel_spmd(nc, in_maps, cores, trace=trace)
    except Exception:
        import time as _t
        _t.sleep(2.0)
        return run_bass_kernel_spmd(nc, in_maps, cores, trace=trace)


def _q8j(a):
    import jax.numpy as jnp
    return jnp.asarray(a).astype(jnp.float8_e4m3fn).astype(jnp.float32)


def kernel(**inputs):
    import jax
    import jax.numpy as jnp
    cpu = jax.devices("cpu")[0]

    x = np.ascontiguousarray(np.asarray(inputs["x"], dtype=np.float32))
    Wq = np.asarray(inputs["Wq"], dtype=np.float32)
    bq = np.asarray(inputs["bq"], dtype=np.float32)
    Wkv_down = np.asarray(inputs["Wkv_down"], dtype=np.float32)
    bkv_down = np.asarray(inputs["bkv_down"], dtype=np.float32)
    Wq_down = np.asarray(inputs["Wq_down"], dtype=np.float32)
    bq_down = np.asarray(inputs["bq_down"], dtype=np.float32)
    Wkv_up = np.asarray(inputs["Wkv_up"], dtype=np.float32)
    bkv_up = np.asarray(inputs["bkv_up"], dtype=np.float32)
    Wout = np.asarray(inputs["Wout"], dtype=np.float32)
    bout = np.asarray(inputs["bout"], dtype=np.float32)
    k = int(np.asarray(inputs["top_k"]))
    assert k == TOPK, f"kernel hardcoded for top_k={TOPK}, got {k}"

    if "l1" not in _STATE:
        _STATE["l1"] = _build_l1()
    if "l2" not in _STATE:
        _STATE["l2"] = _build_l2()

    trace = False  # NTFF profiling hook unavailable under this axon client

    q_last = x[:, -1, :]                                   # [B, D]
    with jax.default_device(cpu):
        # bit-exact replication of the reference's fp8 indexer query + q
        q_idx = np.asarray(_q8j(q_last) @ _q8j(Wq_down) + _q8j(bq_down))
        q = np.asarray(jnp.asarray(q_last) @ jnp.asarray(Wq)) + bq

    # ---------------- launch 1: noisy full-S scores (fp8 GEMV)
    w = np.einsum("dl,bl->bd", Wkv_down[:, :L], q_idx)     # [B, D]
    w8 = w.astype(F8NP)
    in1 = []
    for c in range(NCORES):
        x8s = (x[c].astype(F8NP).reshape(NSB, 512, ND, 128)
               .transpose(0, 3, 2, 1).reshape(NSB * 128, ND * 512))
        in1.append({
            "x8S": np.ascontiguousarray(x8s),
            "w8": np.ascontiguousarray(w8[c].reshape(ND, 128).T),
        })
    r1 = _run_spmd_retry(_STATE["l1"], in1, list(range(NCORES)), trace=trace)
    LAST_EXEC["l1"] = r1
    s_noisy = np.stack([r1.results[c]["sc"].T.flatten()
                        for c in range(NCORES)])

    # ---------------- host: exact top-k set via band rescore (bit-exact)
    sel_all = []
    with jax.default_device(cpu):
        jWdk = jnp.asarray(Wkv_down[:, :L])
        jbkd = jnp.asarray(bkv_down[:L])
        for b in range(B):
            order = np.argsort(-np.maximum(s_noisy[b], 0.0), kind="stable")
            certain = order[:k - MARGIN]
            band = order[k - MARGIN:k + MARGIN]
            Kb = jnp.asarray(x[b][band]) @ jWdk + jbkd
            sb = np.asarray(jnp.einsum(
                "l,sl->s", jnp.asarray(q_idx[b]),
                Kb.astype(jnp.float8_e4m3fn).astype(jnp.float32)))
            sb = np.maximum(sb, 0.0)
            pick = band[np.argsort(-sb, kind="stable")[:k - len(certain)]]
            sel_all.append(np.concatenate([certain, pick]))

    # ---------------- launch 2: absorbed attention over the selected set
    # q~[b,h,l] = (q[b] . Wk_up[:, h-block]) / sqrt(dh), host-exact
    qfold = np.einsum("bhd,lhd->bhl", q.reshape(B, H, DH),
                      Wkv_up[:, :D].reshape(L, H, DH)) * RSQ
    # fused SBUF-layout copies: [p, chunk*W + col] = M[chunk*128 + p, col]
    wdkVc = np.ascontiguousarray(
        Wkv_down[:, L:].reshape(ND, 128, L).transpose(1, 0, 2)
        .reshape(128, ND * L)).astype(F16NP)
    wvupc = np.ascontiguousarray(
        Wkv_up[:, D:].reshape(NLT, 128, D).transpose(1, 0, 2)
        .reshape(128, NLT * D)).astype(F16NP)
    woutc = np.ascontiguousarray(
        Wout.reshape(ND, 128, D).transpose(1, 0, 2)
        .reshape(128, ND * D)).astype(F16NP)
    # out = (o~raw @ Wv_up) @ Wout + (bkv_down_V @ Wv_up + bv_up) @ Wout
    #       + bout  -- all V-side biases are additive constants on o because
    #       the normalized attention weights sum to 1
    bfold = (bkv_down[L:] @ Wkv_up[:, D:] + bkv_up[D:]) @ Wout + bout
    boutcc = np.ascontiguousarray(bfold.reshape(ND, 128).T).astype(np.float32)
    # double absorption: w^[b, d, h] = sum_l Wkv_down_K[d, l] q~[b, h, l]
    whatf = np.einsum("dl,bhl->bdh", Wkv_down[:, :L], qfold)
    in2 = []
    for c in range(NCORES):
        # what[p, dc*16+h] = w^[c, dc*128+p, h]
        whatc = np.ascontiguousarray(
            whatf[c].reshape(ND, 128, H).transpose(1, 0, 2)
            .reshape(128, ND * H)).astype(F16NP)
        # slab-major x_sel: row kb*128+p, col d*512+j = x_sel[kb*512+j, d*128+p]
        xss = (x[c][sel_all[c]].astype(F16NP).reshape(NKB, 512, ND, 128)
               .transpose(0, 3, 2, 1).reshape(NKB * 128, ND * 512))
        in2.append({
            "xselS": np.ascontiguousarray(xss),
            "wdkVf": wdkVc,
            "wvupf": wvupc,
            "woutf": woutc,
            "what": whatc,
            "boutc": boutcc,
        })
    r2 = _run_spmd_retry(_STATE["l2"], in2, list(range(NCORES)), trace=trace)
    LAST_EXEC["l2"] = r2
    out = np.stack([r2.results[c]["outc"].T.flatten()
                    for c in range(NCORES)])
    return out.astype(np.float32)


# revision 34
# speedup vs baseline: 1.1066x; 1.1066x over previous
"""DeepSeek sparse attention (single-query, MQA low-rank KV) on 8 trn2 cores.

Strategy (data-parallel: batch b -> core b):
  Launch 1 (device): indexer scores for the last query, folded to a GEMV:
      w_b = Wkv_down[:, :L] @ q_idx[b]  (host, per batch), both x and w_b
      quantized to fp8 e4m3 -> scores[s] = x8[s,:] . w8  on the PE with
      x8 chunks as the stationary operand and w8 columns as the 1-wide
      moving operand.  x is streamed in s-block slabs so the PE trails
      the DMA by one slab; the fp8 x DMA (16.8 MB) is the whole cost.
  Host: noisy-score top-k certain/band split (margin 768 ranks >2x the
      measured 358-rank worst-case displacement of the fp8 GEMV scores
      vs the reference's fp8-emulated scores); band rows rescored
      bit-exactly vs the reference via jax-CPU slice gemm; exact top-k
      set.  Selection must be exact: one swapped token moves the output
      by ~1.7e-2 relative, right at the tolerance.
  Launch 2 (device): doubly-absorbed MLA attention over the selected
      tokens in fp16.  The K path is folded through the query on the
      host twice: q~[h] = q[h] @ Wk_up[:,h]^T / sqrt(dh) and
      w^[h] = Wkv_down_K @ q~[h], so NEITHER the K down-projection nor
      the K up-projection ever runs on the device -- logits[h,k] are a
      direct contraction w^[h] . x_sel[k] (16 matmuls per k-block).
      The only big GEMM left is the V-latent down-projection
      x_sel @ Wkv_down_V (4 psum banks, no drain stalls).  Softmax runs
      without max-subtraction (logits ~N(0, 0.4), bounded ~+-3); exp for
      k-block i hides under block i+1's V pass.  o via per-head Wv_up,
      out-projection with 1-wide moving columns.  K-path biases shift
      logits per-head uniformly and cancel exactly in softmax; all
      V-side biases are additive constants on o (sum of attn = 1) and
      fold into the host output bias (bkv_down_V @ Wv_up + bv_up) @
      Wout + bout.  All big operands are host-relayouted so each lands
      as one contiguous DMA; x slabs get dedicated buffers and are
      issued dep-free ahead of the 23 us Wout transfer.

Shapes hardcoded: B=8, S=8192, D=2048, H=16, dh=128, L=512, k=2048.
"""
import numpy as np
import ml_dtypes

import concourse.bacc as bacc
import concourse.tile as tile
import concourse.mybir as mybir
from concourse import masks
from concourse.bass_utils import run_bass_kernel_spmd

F8NP = ml_dtypes.float8_e4m3fn
F16NP = np.float16
dt = mybir.dt

B, S, D = 8, 8192, 2048
H, DH, L = 16, 128, 512
TOPK = 2048
MARGIN = 768
NCORES = 8
RSQ = float(1.0 / np.sqrt(np.float32(DH)))  # 1/sqrt(128)

ND = D // 128      # 16 d-chunks
NLT = L // 128     # 4 K-latent l-tiles
NKT = TOPK // 128  # 16 k-tiles
NKB = TOPK // 512  # 4 k-blocks
NST = S // 128     # 64 s-tiles
NSB = S // 512     # 16 s-slabs

_STATE = {}
LAST_EXEC = {}


# ---------------------------------------------------------------- launch 1
def _build_l1():
    """scores[s] = sum_d x8[d, s] * w8[d], fp8, one psum column per s-tile."""
    nc = bacc.Bacc("TRN2", target_bir_lowering=False, debug=False,
                   num_devices=NCORES)
    # x8S is host-relayouted slab-major: row i*128+p, col d*512+s holds
    # x[i*512+s, d*128+p], so slab i is one contiguous [128, 8192] DMA.
    x8S = nc.dram_tensor("x8S", [NSB * 128, ND * 512], dt.float8e4,
                         kind="ExternalInput").ap()
    w8 = nc.dram_tensor("w8", [128, ND], dt.float8e4, kind="ExternalInput").ap()
    sc = nc.dram_tensor("sc", [128, NST], dt.float32, kind="ExternalOutput").ap()

    with tile.TileContext(nc) as tc:
        with (
            tc.tile_pool(name="p", bufs=1) as p,
            tc.tile_pool(name="xp", bufs=1) as xp,
            tc.tile_pool(name="ps", bufs=1, space="PSUM") as psp,
        ):
            w = p.tile([128, ND], dt.float8e4)
            nc.sync.dma_start(w[:], w8)
            pst = psp.tile([128, 512], dt.float32)
            sb = p.tile([128, NST], dt.float32)
            xsl = [None] * NSB
            for i in range(NSB):
                # slab i: all 16 d-chunks of s-columns [i*512, (i+1)*512);
                # the last slab lands in 4 pieces to shorten the tail chain
                xsl[i] = xp.tile([128, ND * 512], dt.float8e4,
                                 tag=f"x{i % 3}", name=f"xsl_{i}")
                if i < NSB - 1:
                    nc.sync.dma_start(xsl[i][:], x8S[i * 128:(i + 1) * 128, :])
                else:
                    for q in range(4):
                        nc.sync.dma_start(
                            xsl[i][:, q * 2048:(q + 1) * 2048],
                            x8S[i * 128:(i + 1) * 128, q * 2048:(q + 1) * 2048])
                for st in range(4):
                    s = i * 4 + st
                    for d in range(ND):
                        nc.tensor.matmul(
                            pst[:, s:s + 1],
                            xsl[i][:, d * 512 + st * 128:
                                    d * 512 + (st + 1) * 128],
                            w[:, d:d + 1], start=(d == 0), stop=(d == ND - 1))
                if i % 4 == 3:
                    # drain per quarter into sb; a single output DMA at the
                    # end (interleaved per-quarter DMAs steal serialized
                    # HWDGE/DMA-engine slots from the x stream, ~1.7 us)
                    q = i // 4
                    nc.scalar.copy(sb[:, q * 16:(q + 1) * 16],
                                   pst[:, q * 16:(q + 1) * 16])
            nc.sync.dma_start(sc, sb[:])
    nc.compile()
    return nc


# ---------------------------------------------------------------- launch 2
def _build_l2():
    f16 = dt.float16
    nc = bacc.Bacc("TRN2", target_bir_lowering=False, debug=False,
                   num_devices=NCORES)
    # all big operands are host-relayouted so each lands as one (or a few)
    # contiguous [128, N] DMAs.  x_sel arrives twice: fp8 in [D-part, k]
    # slab layout for the logits contraction, f16 in [k-part, D] slab
    # layout for the attention application x_bar = attn @ x_sel.
    x8selS = nc.dram_tensor("x8selS", [NKB * 128, ND * 512], dt.float8e4,
                            kind="ExternalInput").ap()
    xselKS = nc.dram_tensor("xselKS", [NKB * 128, 4 * D], f16,
                            kind="ExternalInput").ap()
    wdkVf = nc.dram_tensor("wdkVf", [128, ND * L], f16,
                           kind="ExternalInput").ap()
    wvupf = nc.dram_tensor("wvupf", [128, NLT * D], f16,
                           kind="ExternalInput").ap()
    woutf = nc.dram_tensor("woutf", [128, ND * D], f16,
                           kind="ExternalInput").ap()
    what8 = nc.dram_tensor("what8", [128, ND * H], dt.float8e4,
                           kind="ExternalInput").ap()
    boutc = nc.dram_tensor("boutc", [128, ND], dt.float32,
                           kind="ExternalInput").ap()
    outc = nc.dram_tensor("outc", [128, ND], dt.float32,
                          kind="ExternalOutput").ap()

    with tile.TileContext(nc) as tc:
        with (
            tc.tile_pool(name="cst", bufs=1) as cst,
            tc.tile_pool(name="wp", bufs=1) as wp,
        ):
            w8hsb = cst.tile([128, ND * H], dt.float8e4)
            nc.sync.dma_start(w8hsb[:], what8)
            boutcsb = cst.tile([128, ND], dt.float32)
            nc.sync.dma_start(boutcsb[:], boutc)
            idf32 = cst.tile([16, 16], dt.float32)
            masks.make_identity(nc, idf32[:])
            idf16 = cst.tile([16, 16], f16)
            nc.scalar.copy(idf16[:], idf32[:])

            x8sl = wp.tile([128, NKB * ND * 512], dt.float8e4)
            xks = wp.tile([128, NKT * D], f16)
            wdkVsb = wp.tile([128, ND * L], f16)
            wvupsb = wp.tile([128, NLT * D], f16)
            woutsb = wp.tile([128, ND * D], f16)
            den = cst.tile([16, 1], dt.float32)
            rden = cst.tile([16, 1], dt.float32)
            attnE = cst.tile([16, TOPK], f16)
            attnN = cst.tile([16, TOPK], f16)

            # DMA order = consumption order; wout last (only the final
            # out-projection needs it, right before the output DMA)
            for kb in range(NKB):
                nc.sync.dma_start(
                    x8sl[:, kb * ND * 512:(kb + 1) * ND * 512],
                    x8selS[kb * 128:(kb + 1) * 128, :])
            for kb in range(NKB):
                nc.sync.dma_start(xks[:, kb * 4 * D:(kb + 1) * 4 * D],
                                  xselKS[kb * 128:(kb + 1) * 128, :])
            nc.sync.dma_start(wdkVsb[:], wdkVf)
            nc.sync.dma_start(wvupsb[:], wvupf)
            nc.sync.dma_start(woutsb[:], woutf)

            # ---- fp8 logits + softmax (no max-subtraction; w^ was scaled
            # by 64 on the host to stay in fp8 normal range, the exp's
            # scale argument divides it back out)
            with tc.tile_pool(name="lp", bufs=1, space="PSUM") as lp:
                lps = lp.tile([128, TOPK], dt.float32)
                for kb in range(NKB):
                    for d in range(ND):
                        nc.tensor.matmul(
                            lps[:H, kb * 512:(kb + 1) * 512],
                            w8hsb[:, d * H:(d + 1) * H],
                            x8sl[:, kb * ND * 512 + d * 512:
                                 kb * ND * 512 + (d + 1) * 512],
                            start=(d == 0), stop=(d == ND - 1))
                nc.scalar.activation(attnE[:], lps[:H, :],
                                     mybir.ActivationFunctionType.Exp,
                                     scale=1.0 / 64.0, accum_out=den[:])
            nc.vector.reciprocal(rden[:], den[:])
            nc.vector.tensor_scalar_mul(attnN[:], attnE[:], rden[:])

            # ---- attn^T tiles, then x_barT[d, h] = sum_k x_sel[k, d] attn[h, k]
            attnT = cst.tile([128, NKT * H], f16)
            xbT = cst.tile([128, ND * H], f16)
            with (
                tc.tile_pool(name="tps", bufs=1, space="PSUM") as tps,
                tc.tile_pool(name="xbp", bufs=1, space="PSUM") as xbp,
            ):
                for g in range(4):
                    ptt = tps.tile([128, 64], f16, tag=f"t{g % 2}",
                                   name=f"ptt{g}")
                    for i in range(4):
                        kt = g * 4 + i
                        nc.tensor.matmul(
                            ptt[:, i * 16:(i + 1) * 16],
                            attnN[:, kt * 128:(kt + 1) * 128],
                            idf16[:], is_transpose=True)
                    nc.scalar.copy(attnT[:, g * 64:(g + 1) * 64], ptt[:])
                for dcg in range(4):
                    pxb = xbp.tile([128, 64], dt.float32, tag=f"x{dcg}",
                                   name=f"pxb{dcg}")
                    for dci in range(4):
                        dc = dcg * 4 + dci
                        for kt in range(NKT):
                            nc.tensor.matmul(
                                pxb[:, dci * 16:(dci + 1) * 16],
                                xks[:, kt * D + dc * 128:
                                    kt * D + (dc + 1) * 128],
                                attnT[:, kt * H:(kt + 1) * H],
                                start=(kt == 0), stop=(kt == NKT - 1))
                    nc.scalar.copy(xbT[:, dcg * 64:(dcg + 1) * 64], pxb[:])

            # ---- o~[h, l] = x_bar[h] @ Wkv_down_V  (the V down-projection
            # commutes with the attention sum: 16 matmuls instead of a
            # 2048-token GEMM), then o~^T, o, out-projection
            otsb = cst.tile([128, NLT * H], f16)
            with (
                tc.tile_pool(name="otq", bufs=1, space="PSUM") as otq,
                tc.tile_pool(name="ttp", bufs=1, space="PSUM") as ttp,
            ):
                pot = otq.tile([128, L], dt.float32)
                for d in range(ND):
                    nc.tensor.matmul(
                        pot[:H, :],
                        xbT[:, d * H:(d + 1) * H],
                        wdkVsb[:, d * L:(d + 1) * L],
                        start=(d == 0), stop=(d == ND - 1))
                otilsb = cst.tile([16, L], f16)
                nc.scalar.copy(otilsb[:], pot[:H, :])
                ptt2 = ttp.tile([128, NLT * H], f16)
                for lc in range(NLT):
                    nc.tensor.matmul(
                        ptt2[:, lc * H:(lc + 1) * H],
                        otilsb[:, lc * 128:(lc + 1) * 128],
                        idf16[:], is_transpose=True)
                nc.scalar.copy(otsb[:], ptt2[:])

            # oT[dh,h] = sum_l Wv_up[l, h-block] o~T[l, h], 4 heads per psum
            # tile; the valid column for head h is (h%4)*16 + h.
            oTbig = cst.tile([128, 4 * 64], f16)
            with tc.tile_pool(name="hp", bufs=1, space="PSUM") as hp:
                for hb in range(4):
                    ph = hp.tile([128, 64], dt.float32, tag=f"h{hb % 2}",
                                 name=f"ph{hb}")
                    for i in range(4):
                        h = hb * 4 + i
                        for lc in range(NLT):
                            nc.tensor.matmul(
                                ph[:, i * 16:(i + 1) * 16],
                                wvupsb[:, lc * D + h * DH:
                                       lc * D + (h + 1) * DH],
                                otsb[:, lc * H:(lc + 1) * H],
                                start=(lc == 0), stop=(lc == NLT - 1))
                    nc.scalar.copy(oTbig[:, hb * 64:(hb + 1) * 64], ph[:])

            # ---- out-projection: outc[n%128, n//128] = sum_d o[d] Wout[d,n]
            outcsb = cst.tile([128, ND], dt.float32)
            with tc.tile_pool(name="cp", bufs=1, space="PSUM") as cp:
                pc = cp.tile([128, ND], dt.float32)
                for j in range(ND):
                    for dc in range(ND):
                        col = (dc // 4) * 64 + (dc % 4) * 16 + dc
                        nc.tensor.matmul(
                            pc[:, j:j + 1],
                            woutsb[:, dc * D + j * 128: dc * D + (j + 1) * 128],
                            oTbig[:, col:col + 1],
                            start=(dc == 0), stop=(dc == ND - 1))
                nc.vector.tensor_add(outcsb[:], pc[:], boutcsb[:])
            nc.sync.dma_start(outc, outcsb[:])
    nc.compile()
    return nc


# ---------------------------------------------------------------- timing
def model_time(nc):
    """Cost-model (TimelineSim) estimate in ns for one core."""
    from concourse.timeline_sim import TimelineSim
    return TimelineSim(nc).simulate()


def _run_spmd_retry(nc, in_maps, cores, trace=False):
    """One retry: a previously crashed process can leave the device in a
    transient NRT_EXEC_UNIT_UNRECOVERABLE state that clears on re-run."""
    try:
        return run_bass_kernel_spmd(nc, in_maps, cores, trace=trace)
    except Exception:
        import time as _t
        _t.sleep(2.0)
        return run_bass_kernel_spmd(nc, in_maps, cores, trace=trace)


def _q8j(a):
    import jax.numpy as jnp
    return jnp.asarray(a).astype(jnp.float8_e4m3fn).astype(jnp.float32)


def kernel(**inputs):
    import jax
    import jax.numpy as jnp
    cpu = jax.devices("cpu")[0]

    x = np.ascontiguousarray(np.asarray(inputs["x"], dtype=np.float32))
    Wq = np.asarray(inputs["Wq"], dtype=np.float32)
    bq = np.asarray(inputs["bq"], dtype=np.float32)
    Wkv_down = np.asarray(inputs["Wkv_down"], dtype=np.float32)
    bkv_down = np.asarray(inputs["bkv_down"], dtype=np.float32)
    Wq_down = np.asarray(inputs["Wq_down"], dtype=np.float32)
    bq_down = np.asarray(inputs["bq_down"], dtype=np.float32)
    Wkv_up = np.asarray(inputs["Wkv_up"], dtype=np.float32)
    bkv_up = np.asarray(inputs["bkv_up"], dtype=np.float32)
    Wout = np.asarray(inputs["Wout"], dtype=np.float32)
    bout = np.asarray(inputs["bout"], dtype=np.float32)
    k = int(np.asarray(inputs["top_k"]))
    assert k == TOPK, f"kernel hardcoded for top_k={TOPK}, got {k}"

    if "l1" not in _STATE:
        _STATE["l1"] = _build_l1()
    if "l2" not in _STATE:
        _STATE["l2"] = _build_l2()

    trace = False  # NTFF profiling hook unavailable under this axon client

    q_last = x[:, -1, :]                                   # [B, D]
    with jax.default_device(cpu):
        # bit-exact replication of the reference's fp8 indexer query + q
        q_idx = np.asarray(_q8j(q_last) @ _q8j(Wq_down) + _q8j(bq_down))
        q = np.asarray(jnp.asarray(q_last) @ jnp.asarray(Wq)) + bq

    # ---------------- launch 1: noisy full-S scores (fp8 GEMV)
    w = np.einsum("dl,bl->bd", Wkv_down[:, :L], q_idx)     # [B, D]
    w8 = w.astype(F8NP)
    in1 = []
    for c in range(NCORES):
        x8s = (x[c].astype(F8NP).reshape(NSB, 512, ND, 128)
               .transpose(0, 3, 2, 1).reshape(NSB * 128, ND * 512))
        in1.append({
            "x8S": np.ascontiguousarray(x8s),
            "w8": np.ascontiguousarray(w8[c].reshape(ND, 128).T),
        })
    r1 = _run_spmd_retry(_STATE["l1"], in1, list(range(NCORES)), trace=trace)
    LAST_EXEC["l1"] = r1
    s_noisy = np.stack([r1.results[c]["sc"].T.flatten()
                        for c in range(NCORES)])

    # ---------------- host: exact top-k set via band rescore (bit-exact)
    sel_all = []
    with jax.default_device(cpu):
        jWdk = jnp.asarray(Wkv_down[:, :L])
        jbkd = jnp.asarray(bkv_down[:L])
        for b in range(B):
            order = np.argsort(-np.maximum(s_noisy[b], 0.0), kind="stable")
            certain = order[:k - MARGIN]
            band = order[k - MARGIN:k + MARGIN]
            Kb = jnp.asarray(x[b][band]) @ jWdk + jbkd
            sb = np.asarray(jnp.einsum(
                "l,sl->s", jnp.asarray(q_idx[b]),
                Kb.astype(jnp.float8_e4m3fn).astype(jnp.float32)))
            sb = np.maximum(sb, 0.0)
            pick = band[np.argsort(-sb, kind="stable")[:k - len(certain)]]
            sel_all.append(np.concatenate([certain, pick]))

    # ---------------- launch 2: absorbed attention over the selected set
    # q~[b,h,l] = (q[b] . Wk_up[:, h-block]) / sqrt(dh), host-exact
    qfold = np.einsum("bhd,lhd->bhl", q.reshape(B, H, DH),
                      Wkv_up[:, :D].reshape(L, H, DH)) * RSQ
    # fused SBUF-layout copies: [p, chunk*W + col] = M[chunk*128 + p, col]
    wdkVc = np.ascontiguousarray(
        Wkv_down[:, L:].reshape(ND, 128, L).transpose(1, 0, 2)
        .reshape(128, ND * L)).astype(F16NP)
    wvupc = np.ascontiguousarray(
        Wkv_up[:, D:].reshape(NLT, 128, D).transpose(1, 0, 2)
        .reshape(128, NLT * D)).astype(F16NP)
    woutc = np.ascontiguousarray(
        Wout.reshape(ND, 128, D).transpose(1, 0, 2)
        .reshape(128, ND * D)).astype(F16NP)
    # out = (o~raw @ Wv_up) @ Wout + (bkv_down_V @ Wv_up + bv_up) @ Wout
    #       + bout  -- all V-side biases are additive constants on o because
    #       the normalized attention weights sum to 1
    bfold = (bkv_down[L:] @ Wkv_up[:, D:] + bkv_up[D:]) @ Wout + bout
    boutcc = np.ascontiguousarray(bfold.reshape(ND, 128).T).astype(np.float32)
    # double absorption: w^[b, d, h] = sum_l Wkv_down_K[d, l] q~[b, h, l],
    # shipped fp8 scaled by 64 (w^ elements ~0.008 would land in fp8
    # subnormal range); the device exp divides the scale back out.
    whatf = np.einsum("dl,bhl->bdh", Wkv_down[:, :L], qfold)
    in2 = []
    for c in range(NCORES):
        whatc = np.ascontiguousarray(
            (whatf[c] * 64.0).reshape(ND, 128, H).transpose(1, 0, 2)
            .reshape(128, ND * H)).astype(F8NP)
        xs = x[c][sel_all[c]]
        # fp8 [D-part, k] slabs for logits: row kb*128+p, col d*512+j
        #   = x_sel[kb*512+j, d*128+p]
        x8ss = (xs.astype(F8NP).reshape(NKB, 512, ND, 128)
                .transpose(0, 3, 2, 1).reshape(NKB * 128, ND * 512))
        # f16 [k-part, D] slabs for x_bar: row kb*128+p, col ktl*2048+d
        #   = x_sel[kb*512+ktl*128+p, d]
        xks = (xs.astype(F16NP).reshape(NKB, 4, 128, D)
               .transpose(0, 2, 1, 3).reshape(NKB * 128, 4 * D))
        in2.append({
            "x8selS": np.ascontiguousarray(x8ss),
            "xselKS": np.ascontiguousarray(xks),
            "wdkVf": wdkVc,
            "wvupf": wvupc,
            "woutf": woutc,
            "what8": whatc,
            "boutc": boutcc,
        })
    r2 = _run_spmd_retry(_STATE["l2"], in2, list(range(NCORES)), trace=trace)
    LAST_EXEC["l2"] = r2
    out = np.stack([r2.results[c]["outc"].T.flatten()
                    for c in range(NCORES)])
    return out.astype(np.float32)
